# revision 8
# baseline (speedup 1.0000x reference)
"""Trainium2 Bass kernel for nn_GSS (gumbel-softmax hard sampling + gather).

Math: reference computes, per batch b:
  logits[n] = sum_d src[d,n]*w_src[d] + C_b        (C_b constant over n)
  z[k,n]    = (logits[n] + g[k,n]) / tau,  g = -log(-log(u))
  idx[k]    = argmax_n z[k,n]
  outputs   = columns idx[k] of points / src_embedding (straight-through
              one-hot matmul == gather, up to 1 ulp on the selected entry)

Key reductions used here:
  * C_b (the tgt_embedding term) and tau shift/scale all n equally -> they
    cannot change the argmax -> tgt_embedding & temperature are never read.
  * argmax_n(logits[n] + g[k,n]) == argmax_n( exp(-logits[n]) * ln(u[k,n]) )
    (apply strictly increasing map x -> -exp(-x); note ln(u) < 0), so only
    ONE transcendental pass over the big noise tensor is needed.

Per-core (batch-parallel, 1 batch element per NeuronCore):
  PE   : logits broadcast matvec (w replicated stationary)
  ACT  : s = Exp(-logits) [16K values]; L = Ln(u) streaming
  DVE  : tensor_tensor_reduce  W = L * s_bcast (+ running max), then
         max_index over each 128-keypoint tile -> argmax indices
  GPSIMD: dma_gather of the selected rows from a host-pretransposed
         [N, 192] table (src_emb.T ++ points.T ++ pad)
"""

import sys

if "/opt/trn_rl_repo" not in sys.path:
    sys.path.insert(0, "/opt/trn_rl_repo")

import numpy as np

B, K, N, D = 8, 512, 16384, 128
KT = K // 128          # 4 keypoint tiles per core
CHUNK = 2048           # noise chunk width (free dim)
NCH = N // CHUNK       # 8 chunks per keypoint tile
GW = 192               # gather row width: 128 emb + 3 pts + 61 pad (768B, %256)

_CACHE = {}


def build_bass():
    import concourse.bass as bass
    import concourse.tile as tile
    from concourse import mybir
    from concourse.bacc import Bacc

    f32 = mybir.dt.float32
    i16 = mybir.dt.int16
    u32 = mybir.dt.uint32
    AF = mybir.ActivationFunctionType
    ALU = mybir.AluOpType
    AxisX = mybir.AxisListType.X

    nc = Bacc()
    u_d = nc.dram_tensor("u", [K, N], f32, kind="ExternalInput")
    src_d = nc.dram_tensor("src", [D, N], f32, kind="ExternalInput")
    combT_d = nc.dram_tensor("combT", [N, GW], f32, kind="ExternalInput")
    wrep_d = nc.dram_tensor("wrep", [128, 128], f32, kind="ExternalInput")
    gath_d = nc.dram_tensor("gath", [128, KT, GW], f32, kind="ExternalOutput")

    with tile.TileContext(nc) as tc:
        with (
            tc.tile_pool(name="res", bufs=1) as res,          # resident
            tc.tile_pool(name="stream", bufs=3) as stream,    # u / src chunks
            tc.tile_pool(name="lbuf", bufs=3) as lbuf,        # ln(u) chunks
            tc.tile_pool(name="small", bufs=2) as small,
            tc.tile_pool(name="gout", bufs=2) as gout,
            tc.tile_pool(name="psum", bufs=2, space="PSUM") as psum,
        ):
            s_bc = res.tile([128, N], f32)     # exp(-logits) bcast, resident
            w_t = res.tile([128, N], f32)      # W = ln(u)*s for one k-tile
            wrep = res.tile([128, 128], f32)   # stationary weights
            zeros8 = res.tile([128, 8], f32)

            nc.sync.dma_start(wrep[:], wrep_d[:])
            nc.vector.memset(zeros8[:], 0.0)

            # ---- stage 1: s_bcast = exp(-(w_src . src[:, n])) ----
            for g in range(N // CHUNK):
                sc = stream.tile([128, CHUNK], f32, tag="src")
                nc.sync.dma_start(sc[:], src_d[:, g * CHUNK:(g + 1) * CHUNK])
                pt = psum.tile([128, CHUNK], f32)
                for j in range(CHUNK // 512):
                    nc.tensor.matmul(
                        pt[:, j * 512:(j + 1) * 512],
                        wrep[:],
                        sc[:, j * 512:(j + 1) * 512],
                        start=True, stop=True,
                    )
                # s = exp(-logits): ACT evicts PSUM -> SBUF with scale=-1
                nc.scalar.activation(
                    s_bc[:, g * CHUNK:(g + 1) * CHUNK], pt[:], AF.Exp, scale=-1.0
                )

            # ---- stage 2: per keypoint tile ----
            for t in range(KT):
                for c in range(NCH):
                    ut = stream.tile([128, CHUNK], f32, tag="u")
                    nc.sync.dma_start(
                        ut[:], u_d[t * 128:(t + 1) * 128, c * CHUNK:(c + 1) * CHUNK]
                    )
                    lt = lbuf.tile([128, CHUNK], f32, tag="l")
                    nc.scalar.activation(lt[:], ut[:], AF.Ln)
                    nc.vector.tensor_tensor(
                        w_t[:, c * CHUNK:(c + 1) * CHUNK],
                        lt[:],
                        s_bc[:, c * CHUNK:(c + 1) * CHUNK],
                        ALU.mult,
                    )
                # global max per partition, broadcast into 8 lanes, find index
                gmax = small.tile([128, 1], f32, tag="gmax")
                nc.vector.tensor_reduce(gmax[:], w_t[:], AxisX, ALU.max)
                inmax = small.tile([128, 8], f32, tag="inmax")
                nc.vector.tensor_scalar_add(inmax[:], zeros8[:], gmax[:])
                idx8 = small.tile([128, 8], u32, tag="idx8")
                nc.vector.max_index(idx8[:], inmax[:], w_t[:])

                idx16 = small.tile([128, 1], i16, tag="idx16")
                nc.vector.tensor_copy(idx16[:], idx8[:, 0:1])
                # wrap into dma_gather layout: idx j at (partition j%16, free j//16)
                widx = small.tile([128, 8], i16, tag="widx")
                for s in range(8):
                    nc.sync.dma_start(
                        widx[0:16, s:s + 1], idx16[16 * s:16 * (s + 1), 0:1]
                    )
                for g in range(1, 8):
                    nc.sync.dma_start(widx[16 * g:16 * (g + 1), :], widx[0:16, :])

                gt = gout.tile([128, 1, GW], f32, tag="g")
                nc.gpsimd.dma_gather(
                    out_ap=gt[:],
                    in_ap=combT_d[:],
                    idxs_ap=widx[:],
                    num_idxs=128,
                    num_idxs_reg=128,
                    elem_size=GW,
                )
                nc.sync.dma_start(gath_d[:, t:t + 1, :], gt[:])

    nc.finalize()
    return nc


def _get_nc():
    if "nc" not in _CACHE:
        _CACHE["nc"] = build_bass()
    return _CACHE["nc"]


def _make_in_maps(points, src, conv_w, u):
    wrep = np.ascontiguousarray(
        np.repeat(conv_w[:D].astype(np.float32)[:, None], 128, axis=1)
    )
    in_maps = []
    for b in range(B):
        combT = np.zeros((N, GW), np.float32)
        combT[:, :D] = src[b].T
        combT[:, D:D + 3] = points[b].T
        in_maps.append(
            {
                "u": np.ascontiguousarray(u[b], np.float32),
                "src": np.ascontiguousarray(src[b], np.float32),
                "combT": combT,
                "wrep": wrep,
            }
        )
    return in_maps


def _run(inputs, trace=False, **kw):
    from concourse.bass_utils import run_bass_kernel_spmd

    points = np.asarray(inputs["points"], np.float32)
    src = np.asarray(inputs["src_embedding"], np.float32)
    conv_w = np.asarray(inputs["conv_w"], np.float32)
    u = np.asarray(inputs["gumbel_noise"], np.float32)

    nc = _get_nc()
    in_maps = _make_in_maps(points, src, conv_w, u)
    res = run_bass_kernel_spmd(nc, in_maps, core_ids=list(range(B)),
                               trace=trace, **kw)

    new_points = np.empty((B, 3, K), np.float32)
    new_emb = np.empty((B, D, K), np.float32)
    for b in range(B):
        g = np.asarray(res.results[b]["gath"])        # [128, KT, GW]
        gk = np.transpose(g, (1, 0, 2)).reshape(K, GW)  # k = t*128 + p
        new_emb[b] = gk[:, :D].T
        new_points[b] = gk[:, D:D + 3].T
    return (new_points, new_emb), res


def kernel(**inputs):
    out, _ = _run(inputs)
    return out
